# revision 3
# baseline (speedup 1.0000x reference)
"""MoE layer (8 experts, top-2) on 8 TRN2 NeuronCores, expert-parallel.

Host does the router + dispatch/combine (all-to-all equivalent); each core
runs the two FFN matmuls for one expert on its gathered tokens using fp32r
(tf32-like) matmuls on the PE array.

Self-contained: hardcodes shapes B=4, S=2048, HIDDEN=1024, INNER=2048,
NUM_EXPERTS=8, TOP_K=2.
"""

import sys

import numpy as np

try:
    import concourse.bass as bass  # noqa: F401
except ImportError:
    sys.path.insert(0, "/opt/trn_rl_repo")

import concourse.tile as tile
from concourse import bacc, mybir
from concourse.bass_utils import run_bass_kernel_spmd

H = 1024
INNER = 2048
E = 8
TOP_K = 2
N_D = H // 128  # 8 k-tiles for matmul1
N_I = INNER // 128  # 16 k-tiles for matmul2
TCH = 512  # token chunk (moving free dim)

F32 = mybir.dt.float32
F32R = mybir.dt.float32r
RELU = mybir.ActivationFunctionType.Relu

# test.py hooks: set TRACE=True before calling kernel() to profile;
# LAST_RESULT then holds the BassKernelResults (exec_time_ns etc.).
TRACE = False
TRACE_KWARGS = {}
LAST_RESULT = None

_cache = {}


def _chunks_of(c):
    full, rem = divmod(c, TCH)
    return [TCH] * full + ([rem] if rem else [])


def _build(c):
    nc = bacc.Bacc("TRN2", target_bir_lowering=False, debug=False, num_devices=8)

    xT = nc.dram_tensor("xT", [H, c], F32R, kind="ExternalInput")
    w1 = nc.dram_tensor("w1", [H, INNER], F32R, kind="ExternalInput")
    w2 = nc.dram_tensor("w2", [INNER, H], F32R, kind="ExternalInput")
    b1r = nc.dram_tensor("b1r", [128, N_I], F32, kind="ExternalInput")
    b2r = nc.dram_tensor("b2r", [1, H], F32R, kind="ExternalInput")
    wv = nc.dram_tensor("wv", [128, c // 128], F32, kind="ExternalInput")
    onev = nc.dram_tensor("onev", [1, 128], F32R, kind="ExternalInput")
    y = nc.dram_tensor("y", [c, H], F32, kind="ExternalOutput")

    with tile.TileContext(nc) as tc:
        with (
            tc.tile_pool(name="weights", bufs=1) as wpool,
            tc.tile_pool(name="tokens", bufs=2) as tpool,
            tc.tile_pool(name="hidden", bufs=1) as hpool,
            tc.tile_pool(name="out", bufs=4) as opool,
            tc.tile_pool(name="psumA", bufs=4, space="PSUM") as psA,
            tc.tile_pool(name="psumB", bufs=4, space="PSUM") as psB,
        ):
            w1_sb = []
            for d in range(N_D):
                t = wpool.tile([128, INNER], F32R, tag=f"w1_{d}")
                nc.sync.dma_start(t[:], w1.ap()[d * 128:(d + 1) * 128, :])
                w1_sb.append(t)
            w2_sb = []
            for i in range(N_I):
                t = wpool.tile([128, H], F32R, tag=f"w2_{i}")
                nc.sync.dma_start(t[:], w2.ap()[i * 128:(i + 1) * 128, :])
                w2_sb.append(t)
            b1_sb = wpool.tile([128, N_I], F32, tag="b1")
            nc.sync.dma_start(b1_sb[:], b1r.ap())
            b2_sb = wpool.tile([1, H], F32R, tag="b2")
            nc.sync.dma_start(b2_sb[:], b2r.ap())
            wv_sb = wpool.tile([128, c // 128], F32, tag="wv")
            nc.sync.dma_start(wv_sb[:], wv.ap())
            ones = wpool.tile([1, 128], F32R, tag="ones")
            nc.sync.dma_start(ones[:], onev.ap())

            off = 0
            for tc_sz in _chunks_of(c):
                ng = tc_sz // 128
                tt = tpool.tile([128, N_D * TCH], F32R, tag="T")
                for d in range(N_D):
                    nc.sync.dma_start(
                        tt[:, d * tc_sz:(d + 1) * tc_sz],
                        xT.ap()[d * 128:(d + 1) * 128, off:off + tc_sz],
                    )
                hh = hpool.tile([128, N_I * TCH], F32R, tag="h")
                for i in range(N_I):
                    pa = psA.tile([128, TCH], F32)
                    for d in range(N_D):
                        nc.tensor.matmul(
                            pa[:, :tc_sz],
                            w1_sb[d][:, i * 128:(i + 1) * 128],
                            tt[:, d * tc_sz:(d + 1) * tc_sz],
                            start=(d == 0),
                            stop=(d == N_D - 1),
                        )
                    nc.scalar.activation(
                        hh[:, i * tc_sz:(i + 1) * tc_sz],
                        pa[:, :tc_sz],
                        RELU,
                        bias=b1_sb[:, i:i + 1],
                    )
                for ts in range(ng):
                    g = off // 128 + ts
                    for dc in range(2):
                        pb = psB.tile([128, 512], F32)
                        for i in range(N_I):
                            nc.tensor.matmul(
                                pb[:],
                                hh[:, i * tc_sz + ts * 128:i * tc_sz + (ts + 1) * 128],
                                w2_sb[i][:, dc * 512:(dc + 1) * 512],
                                start=(i == 0),
                                stop=False,
                            )
                        nc.tensor.matmul(
                            pb[:],
                            ones[:, :],
                            b2_sb[:, dc * 512:(dc + 1) * 512],
                            start=False,
                            stop=True,
                        )
                        oo = opool.tile([128, 512], F32, tag="o")
                        nc.vector.tensor_scalar_mul(oo[:], pb[:], wv_sb[:, g:g + 1])
                        nc.sync.dma_start(
                            y.ap()[g * 128:(g + 1) * 128, dc * 512:(dc + 1) * 512],
                            oo[:],
                        )
                off += tc_sz

    nc.compile()
    return nc


def kernel(x, Wr, br, W1, b1, W2, b2):
    global LAST_RESULT
    x = np.asarray(x, dtype=np.float32)
    Wr = np.asarray(Wr, dtype=np.float32)
    br = np.asarray(br, dtype=np.float32)
    W1 = np.asarray(W1, dtype=np.float32)
    b1 = np.asarray(b1, dtype=np.float32)
    W2 = np.asarray(W2, dtype=np.float32)
    b2 = np.asarray(b2, dtype=np.float32)

    batch, seq, hidden = x.shape
    x2d = x.reshape(-1, hidden)
    n = x2d.shape[0]

    # Router (matches jax reference: top-2 descending, stable ties, softmax).
    logits = x2d @ Wr + br
    order = np.argsort(-logits, axis=1, kind="stable")[:, :TOP_K]
    l0 = logits[np.arange(n), order[:, 0]]
    l1 = logits[np.arange(n), order[:, 1]]
    e1 = np.exp(l1 - l0)
    denom = 1.0 + e1
    top_w = np.stack([1.0 / denom, e1 / denom], axis=1).astype(np.float32)

    rows_l, wsel_l = [], []
    for e in range(E):
        rows, cols = np.nonzero(order == e)
        rows_l.append(rows)
        wsel_l.append(top_w[rows, cols])
    counts = np.array([len(r) for r in rows_l])

    c = max(256, int(-(-counts.max() // 128)) * 128)

    if c not in _cache:
        _cache[c] = _build(c)
    nc = _cache[c]

    in_maps = []
    for e in range(E):
        rows = rows_l[e]
        ne = len(rows)
        xTe = np.zeros((H, c), dtype=np.float32)
        xTe[:, :ne] = x2d[rows].T
        wve = np.zeros(c, dtype=np.float32)
        wve[:ne] = wsel_l[e]
        in_maps.append(
            {
                "xT": xTe,
                "w1": np.ascontiguousarray(W1[e]),
                "w2": np.ascontiguousarray(W2[e]),
                "b1r": np.ascontiguousarray(b1[e].reshape(N_I, 128).T),
                "b2r": b2[e].reshape(1, H).copy(),
                "wv": np.ascontiguousarray(wve.reshape(-1, 128).T),
                "onev": np.ones((1, 128), dtype=np.float32),
            }
        )

    res = run_bass_kernel_spmd(
        nc, in_maps, list(range(E)), trace=TRACE, **TRACE_KWARGS
    )
    LAST_RESULT = res

    out = np.zeros((n, hidden), dtype=np.float32)
    for e in range(E):
        rows = rows_l[e]
        out[rows] += res.results[e]["y"][: len(rows)]
    return out.reshape(batch, seq, hidden)


# revision 5
# speedup vs baseline: 1.0797x; 1.0797x over previous
"""MoE layer (8 experts, top-2) on 8 TRN2 NeuronCores, expert-parallel.

Host does the router + dispatch/combine (all-to-all equivalent); each core
runs the two FFN matmuls for one expert on its gathered tokens using fp32r
(tf32-like) matmuls on the PE array.

Self-contained: hardcodes shapes B=4, S=2048, HIDDEN=1024, INNER=2048,
NUM_EXPERTS=8, TOP_K=2.
"""

import sys

import numpy as np

try:
    import concourse.bass as bass  # noqa: F401
except ImportError:
    sys.path.insert(0, "/opt/trn_rl_repo")

import concourse.tile as tile
from concourse import bacc, mybir
from concourse.bass_utils import run_bass_kernel_spmd

H = 1024
INNER = 2048
E = 8
TOP_K = 2
N_D = H // 128  # 8 k-tiles for matmul1
N_I = INNER // 128  # 16 k-tiles for matmul2
TCH = 512  # token chunk (moving free dim)

F32 = mybir.dt.float32
F32R = mybir.dt.float32r
RELU = mybir.ActivationFunctionType.Relu

# test.py hooks: set TRACE=True before calling kernel() to profile;
# LAST_RESULT then holds the BassKernelResults (exec_time_ns etc.).
TRACE = False
TRACE_KWARGS = {}
LAST_RESULT = None

_cache = {}


def _chunks_of(c):
    full, rem = divmod(c, TCH)
    return [TCH] * full + ([rem] if rem else [])


def _build(c):
    nc = bacc.Bacc("TRN2", target_bir_lowering=False, debug=False, num_devices=8)

    xT = nc.dram_tensor("xT", [H, c], F32R, kind="ExternalInput")
    w1 = nc.dram_tensor("w1", [H, INNER], F32R, kind="ExternalInput")
    w2 = nc.dram_tensor("w2", [INNER, H], F32R, kind="ExternalInput")
    b1r = nc.dram_tensor("b1r", [128, N_I], F32, kind="ExternalInput")
    b2r = nc.dram_tensor("b2r", [1, H], F32R, kind="ExternalInput")
    wv = nc.dram_tensor("wv", [128, c // 128], F32, kind="ExternalInput")
    onev = nc.dram_tensor("onev", [1, 128], F32R, kind="ExternalInput")
    y = nc.dram_tensor("y", [c, H], F32, kind="ExternalOutput")

    with tile.TileContext(nc) as tc:
        with (
            tc.tile_pool(name="weights", bufs=1) as wpool,
            tc.tile_pool(name="tokens", bufs=2) as tpool,
            tc.tile_pool(name="hidden", bufs=1) as hpool,
            tc.tile_pool(name="out", bufs=4) as opool,
            tc.tile_pool(name="psumA", bufs=4, space="PSUM") as psA,
            tc.tile_pool(name="psumB", bufs=4, space="PSUM") as psB,
        ):
            # Tiny constants first (13KB): needed by the first relu/scale.
            b1_sb = wpool.tile([128, N_I], F32, tag="b1")
            nc.sync.dma_start(b1_sb[:], b1r.ap())
            b2_sb = wpool.tile([1, H], F32R, tag="b2")
            nc.sync.dma_start(b2_sb[:], b2r.ap())
            wv_sb = wpool.tile([128, c // 128], F32, tag="wv")
            nc.sync.dma_start(wv_sb[:], wv.ap())
            ones = wpool.tile([1, 128], F32R, tag="ones")
            nc.sync.dma_start(ones[:], onev.ap())

            chunk_sizes = _chunks_of(c)

            # Interleave W1 d-tiles with chunk-0 token d-slices so the first
            # stage-A accumulation group can start as soon as possible; W2
            # (only needed once stage B of chunk 0 starts, ~27us in) is
            # DMA'd after chunk-0 stage A is issued.
            tc0 = chunk_sizes[0]
            tt0 = tpool.tile([128, N_D * TCH], F32R, tag="T")
            w1_sb = []
            for d in range(N_D):
                t = wpool.tile([128, INNER], F32R, tag=f"w1_{d}")
                nc.sync.dma_start(t[:], w1.ap()[d * 128:(d + 1) * 128, :])
                w1_sb.append(t)
                nc.sync.dma_start(
                    tt0[:, d * tc0:(d + 1) * tc0],
                    xT.ap()[d * 128:(d + 1) * 128, 0:tc0],
                )

            w2_sb = [None] * N_I

            def _load_w2():
                for i in range(N_I):
                    t = wpool.tile([128, H], F32R, tag=f"w2_{i}")
                    nc.sync.dma_start(t[:], w2.ap()[i * 128:(i + 1) * 128, :])
                    w2_sb[i] = t

            off = 0
            for ci, tc_sz in enumerate(chunk_sizes):
                ng = tc_sz // 128
                if ci == 0:
                    tt = tt0
                else:
                    tt = tpool.tile([128, N_D * TCH], F32R, tag="T")
                    for d in range(N_D):
                        nc.sync.dma_start(
                            tt[:, d * tc_sz:(d + 1) * tc_sz],
                            xT.ap()[d * 128:(d + 1) * 128, off:off + tc_sz],
                        )
                hh = hpool.tile([128, N_I * TCH], F32R, tag="h")
                for i in range(N_I):
                    pa = psA.tile([128, TCH], F32)
                    for d in range(N_D):
                        nc.tensor.matmul(
                            pa[:, :tc_sz],
                            w1_sb[d][:, i * 128:(i + 1) * 128],
                            tt[:, d * tc_sz:(d + 1) * tc_sz],
                            start=(d == 0),
                            stop=(d == N_D - 1),
                        )
                    nc.scalar.activation(
                        hh[:, i * tc_sz:(i + 1) * tc_sz],
                        pa[:, :tc_sz],
                        RELU,
                        bias=b1_sb[:, i:i + 1],
                    )
                if ci == 0:
                    _load_w2()
                for ts in range(ng):
                    g = off // 128 + ts
                    for dc in range(2):
                        pb = psB.tile([128, 512], F32)
                        for i in range(N_I):
                            nc.tensor.matmul(
                                pb[:],
                                hh[:, i * tc_sz + ts * 128:i * tc_sz + (ts + 1) * 128],
                                w2_sb[i][:, dc * 512:(dc + 1) * 512],
                                start=(i == 0),
                                stop=False,
                            )
                        nc.tensor.matmul(
                            pb[:],
                            ones[:, :],
                            b2_sb[:, dc * 512:(dc + 1) * 512],
                            start=False,
                            stop=True,
                        )
                        oo = opool.tile([128, 512], F32, tag="o")
                        nc.vector.tensor_scalar_mul(oo[:], pb[:], wv_sb[:, g:g + 1])
                        nc.sync.dma_start(
                            y.ap()[g * 128:(g + 1) * 128, dc * 512:(dc + 1) * 512],
                            oo[:],
                        )
                off += tc_sz

    nc.compile()
    return nc


def kernel(x, Wr, br, W1, b1, W2, b2):
    global LAST_RESULT
    x = np.asarray(x, dtype=np.float32)
    Wr = np.asarray(Wr, dtype=np.float32)
    br = np.asarray(br, dtype=np.float32)
    W1 = np.asarray(W1, dtype=np.float32)
    b1 = np.asarray(b1, dtype=np.float32)
    W2 = np.asarray(W2, dtype=np.float32)
    b2 = np.asarray(b2, dtype=np.float32)

    batch, seq, hidden = x.shape
    x2d = x.reshape(-1, hidden)
    n = x2d.shape[0]

    # Router (matches jax reference: top-2 descending, stable ties, softmax).
    logits = x2d @ Wr + br
    order = np.argsort(-logits, axis=1, kind="stable")[:, :TOP_K]
    l0 = logits[np.arange(n), order[:, 0]]
    l1 = logits[np.arange(n), order[:, 1]]
    e1 = np.exp(l1 - l0)
    denom = 1.0 + e1
    top_w = np.stack([1.0 / denom, e1 / denom], axis=1).astype(np.float32)

    rows_l, wsel_l = [], []
    for e in range(E):
        rows, cols = np.nonzero(order == e)
        rows_l.append(rows)
        wsel_l.append(top_w[rows, cols])
    counts = np.array([len(r) for r in rows_l])

    c = max(256, int(-(-counts.max() // 128)) * 128)

    if c not in _cache:
        _cache[c] = _build(c)
    nc = _cache[c]

    in_maps = []
    for e in range(E):
        rows = rows_l[e]
        ne = len(rows)
        xTe = np.zeros((H, c), dtype=np.float32)
        xTe[:, :ne] = x2d[rows].T
        wve = np.zeros(c, dtype=np.float32)
        wve[:ne] = wsel_l[e]
        in_maps.append(
            {
                "xT": xTe,
                "w1": np.ascontiguousarray(W1[e]),
                "w2": np.ascontiguousarray(W2[e]),
                "b1r": np.ascontiguousarray(b1[e].reshape(N_I, 128).T),
                "b2r": b2[e].reshape(1, H).copy(),
                "wv": np.ascontiguousarray(wve.reshape(-1, 128).T),
                "onev": np.ones((1, 128), dtype=np.float32),
            }
        )

    res = run_bass_kernel_spmd(
        nc, in_maps, list(range(E)), trace=TRACE, **TRACE_KWARGS
    )
    LAST_RESULT = res

    out = np.zeros((n, hidden), dtype=np.float32)
    for e in range(E):
        rows = rows_l[e]
        out[rows] += res.results[e]["y"][: len(rows)]
    return out.reshape(batch, seq, hidden)


# revision 7
# speedup vs baseline: 1.2390x; 1.1476x over previous
"""MoE layer (8 experts, top-2) on 8 TRN2 NeuronCores, expert-parallel.

Host does the router + dispatch/combine (all-to-all equivalent); each core
runs the two FFN matmuls for one expert on its gathered tokens using fp32r
(tf32-like) matmuls on the PE array. The per-expert output bias b2 is
applied during the host combine (y_dev = w * (relu(x@W1+b1) @ W2), host
adds w*b2).

Self-contained: hardcodes shapes HIDDEN=1024, INNER=2048, NUM_EXPERTS=8,
TOP_K=2.
"""

import sys

import numpy as np

try:
    import concourse.bass as bass  # noqa: F401
except ImportError:
    sys.path.insert(0, "/opt/trn_rl_repo")

import concourse.tile as tile
from concourse import bacc, mybir
from concourse.bass_utils import run_bass_kernel_spmd

H = 1024
INNER = 2048
E = 8
TOP_K = 2
N_D = H // 128  # 8 k-tiles for matmul1
N_I = INNER // 128  # 16 k-tiles for matmul2
TCH = 512  # token chunk (moving free dim)

F32 = mybir.dt.float32
F32R = mybir.dt.float32r
RELU = mybir.ActivationFunctionType.Relu

# test.py hooks: set TRACE=True before calling kernel() to profile;
# LAST_RESULT then holds the BassKernelResults (exec_time_ns etc.).
TRACE = False
TRACE_KWARGS = {}
LAST_RESULT = None

_cache = {}


def _chunks_of(c):
    full, rem = divmod(c, TCH)
    return [TCH] * full + ([rem] if rem else [])


def _build(c):
    nc = bacc.Bacc("TRN2", target_bir_lowering=False, debug=False, num_devices=8)

    xT = nc.dram_tensor("xT", [H, c], F32R, kind="ExternalInput")
    # W1 pre-tiled on host into inner-dim slabs: w1t[i][p, d*128+m] =
    # W1[d*128+p, i*128+m], so stage A's k-group i needs only slab i.
    w1 = nc.dram_tensor("w1t", [N_I, 128, H], F32R, kind="ExternalInput")
    w2 = nc.dram_tensor("w2", [INNER, H], F32R, kind="ExternalInput")
    b1r = nc.dram_tensor("b1r", [128, N_I], F32, kind="ExternalInput")
    wv = nc.dram_tensor("wv", [128, c // 128], F32, kind="ExternalInput")
    y = nc.dram_tensor("y", [c, H], F32, kind="ExternalOutput")

    with tile.TileContext(nc) as tc:
        with (
            tc.tile_pool(name="weights", bufs=1) as wpool,
            tc.tile_pool(name="tokens", bufs=2) as tpool,
            tc.tile_pool(name="hidden", bufs=1) as hpool,
            tc.tile_pool(name="out", bufs=4) as opool,
            tc.tile_pool(name="psumA", bufs=4, space="PSUM") as psA,
            tc.tile_pool(name="psumB", bufs=4, space="PSUM") as psB,
        ):
            # Tiny constants first (13KB): needed by the first relu/scale.
            b1_sb = wpool.tile([128, N_I], F32, tag="b1")
            nc.sync.dma_start(b1_sb[:], b1r.ap())
            wv_sb = wpool.tile([128, c // 128], F32, tag="wv")
            nc.sync.dma_start(wv_sb[:], wv.ap())

            chunk_sizes = _chunks_of(c)

            # Chunk-0 tokens + W1 slabs stream in; stage A group i only
            # needs slab i, so the PE can start after ~2.5MB of DMA.
            tc0 = chunk_sizes[0]
            tt0 = tpool.tile([128, N_D * TCH], F32R, tag="T")
            for d in range(N_D):
                nc.sync.dma_start(
                    tt0[:, d * tc0:(d + 1) * tc0],
                    xT.ap()[d * 128:(d + 1) * 128, 0:tc0],
                )
            w1_sb = []
            for i in range(N_I):
                t = wpool.tile([128, H], F32R, tag=f"w1_{i}")
                nc.sync.dma_start(t[:], w1.ap()[i])
                w1_sb.append(t)
            # W2 slabs follow; chunk-0 stage B consumes them i-outer so it
            # can start on slab 0 instead of waiting for all 8MB.
            w2_sb = []
            for i in range(N_I):
                t = wpool.tile([128, H], F32R, tag=f"w2_{i}")
                nc.sync.dma_start(t[:], w2.ap()[i * 128:(i + 1) * 128, :])
                w2_sb.append(t)

            def stage_a(tt, hh, tc_sz):
                for i in range(N_I):
                    pa = psA.tile([128, TCH], F32, tag="pa")
                    for d in range(N_D):
                        nc.tensor.matmul(
                            pa[:, :tc_sz],
                            w1_sb[i][:, d * 128:(d + 1) * 128],
                            tt[:, d * tc_sz:(d + 1) * tc_sz],
                            start=(d == 0),
                            stop=(d == N_D - 1),
                        )
                    nc.scalar.activation(
                        hh[:, i * tc_sz:(i + 1) * tc_sz],
                        pa[:, :tc_sz],
                        RELU,
                        bias=b1_sb[:, i:i + 1],
                    )

            def emit_out(pb, g, dc):
                oo = opool.tile([128, 512], F32, tag="o")
                nc.vector.tensor_scalar_mul(oo[:], pb[:], wv_sb[:, g:g + 1])
                nc.sync.dma_start(
                    y.ap()[g * 128:(g + 1) * 128, dc * 512:(dc + 1) * 512],
                    oo[:],
                )

            def stage_b_iouter(hh, tc_sz, off):
                ng = tc_sz // 128
                for ts0 in range(0, ng, 2):
                    nts = min(2, ng - ts0)
                    pbs = {}
                    for ts in range(ts0, ts0 + nts):
                        for dc in range(2):
                            pbs[ts, dc] = psB.tile([128, 512], F32, tag="pb", name=f"pb_{ts}_{dc}")
                    for i in range(N_I):
                        for ts in range(ts0, ts0 + nts):
                            lhsT = hh[:, i * tc_sz + ts * 128:i * tc_sz + (ts + 1) * 128]
                            for dc in range(2):
                                nc.tensor.matmul(
                                    pbs[ts, dc][:],
                                    lhsT,
                                    w2_sb[i][:, dc * 512:(dc + 1) * 512],
                                    start=(i == 0),
                                    stop=(i == N_I - 1),
                                )
                    for ts in range(ts0, ts0 + nts):
                        for dc in range(2):
                            emit_out(pbs[ts, dc], off // 128 + ts, dc)

            def stage_b(hh, tc_sz, off):
                ng = tc_sz // 128
                for ts in range(ng):
                    g = off // 128 + ts
                    for dc in range(2):
                        pb = psB.tile([128, 512], F32, tag="pb")
                        for i in range(N_I):
                            nc.tensor.matmul(
                                pb[:],
                                hh[:, i * tc_sz + ts * 128:i * tc_sz + (ts + 1) * 128],
                                w2_sb[i][:, dc * 512:(dc + 1) * 512],
                                start=(i == 0),
                                stop=(i == N_I - 1),
                            )
                        emit_out(pb, g, dc)

            off = 0
            for ci, tc_sz in enumerate(chunk_sizes):
                if ci == 0:
                    tt = tt0
                else:
                    tt = tpool.tile([128, N_D * TCH], F32R, tag="T")
                    for d in range(N_D):
                        nc.sync.dma_start(
                            tt[:, d * tc_sz:(d + 1) * tc_sz],
                            xT.ap()[d * 128:(d + 1) * 128, off:off + tc_sz],
                        )
                hh = hpool.tile([128, N_I * TCH], F32R, tag="h")
                stage_a(tt, hh, tc_sz)
                if ci == 0:
                    stage_b_iouter(hh, tc_sz, off)
                else:
                    stage_b(hh, tc_sz, off)
                off += tc_sz

    nc.compile()
    return nc


def kernel(x, Wr, br, W1, b1, W2, b2):
    global LAST_RESULT
    x = np.asarray(x, dtype=np.float32)
    Wr = np.asarray(Wr, dtype=np.float32)
    br = np.asarray(br, dtype=np.float32)
    W1 = np.asarray(W1, dtype=np.float32)
    b1 = np.asarray(b1, dtype=np.float32)
    W2 = np.asarray(W2, dtype=np.float32)
    b2 = np.asarray(b2, dtype=np.float32)

    batch, seq, hidden = x.shape
    x2d = x.reshape(-1, hidden)
    n = x2d.shape[0]

    # Router (matches jax reference: top-2 descending, stable ties, softmax).
    logits = x2d @ Wr + br
    order = np.argsort(-logits, axis=1, kind="stable")[:, :TOP_K]
    l0 = logits[np.arange(n), order[:, 0]]
    l1 = logits[np.arange(n), order[:, 1]]
    e1 = np.exp(l1 - l0)
    denom = 1.0 + e1
    top_w = np.stack([1.0 / denom, e1 / denom], axis=1).astype(np.float32)

    rows_l, wsel_l = [], []
    for e in range(E):
        rows, cols = np.nonzero(order == e)
        rows_l.append(rows)
        wsel_l.append(top_w[rows, cols])
    counts = np.array([len(r) for r in rows_l])

    c = max(256, int(-(-counts.max() // 128)) * 128)

    if c not in _cache:
        _cache[c] = _build(c)
    nc = _cache[c]

    in_maps = []
    for e in range(E):
        rows = rows_l[e]
        ne = len(rows)
        xTe = np.zeros((H, c), dtype=np.float32)
        xTe[:, :ne] = x2d[rows].T
        wve = np.zeros(c, dtype=np.float32)
        wve[:ne] = wsel_l[e]
        w1t = np.ascontiguousarray(
            W1[e].reshape(N_D, 128, N_I, 128).transpose(2, 1, 0, 3).reshape(N_I, 128, H)
        )
        in_maps.append(
            {
                "xT": xTe,
                "w1t": w1t,
                "w2": np.ascontiguousarray(W2[e]),
                "b1r": np.ascontiguousarray(b1[e].reshape(N_I, 128).T),
                "wv": np.ascontiguousarray(wve.reshape(-1, 128).T),
            }
        )

    res = run_bass_kernel_spmd(
        nc, in_maps, list(range(E)), trace=TRACE, **TRACE_KWARGS
    )
    LAST_RESULT = res

    out = np.zeros((n, hidden), dtype=np.float32)
    for e in range(E):
        rows = rows_l[e]
        ne = len(rows)
        # device returned w*(relu(x@W1+b1)@W2); add w*b2 here
        out[rows] += res.results[e]["y"][:ne] + wsel_l[e][:, None] * b2[e][None, :]
    return out.reshape(batch, seq, hidden)
